# revision 5
# baseline (speedup 1.0000x reference)
"""Multi-head attention (softmax over the QUERY axis) on 8 TRN2 NeuronCores.

Sharding: tensor-parallel over heads. Core c computes heads {2c, 2c+1} for
both batches (Q/K/V projection + attention), AllGathers the concatenated
head outputs (bf16), then computes a 128-wide slice of the WO projection
(output-feature sharded via per-core weight slices, so the SPMD program is
identical across cores). Host concatenates the 8 e-slices.

All matmuls run in bf16 with fp32 PSUM accumulation. Softmax over queries
(axis=2 of [B,H,Lq,Lk]) is computed on S^T tiles ([key-partition,
query-free]) so the reduction is along the free axis; max-subtraction is
skipped (|S*scale| <= ~6, exp is safe in fp32) and the per-key 1/sum is
folded into V rows before the P@V matmul.
"""

import sys

for _p in ("/opt/trn_rl_repo",):
    if _p not in sys.path:
        sys.path.insert(0, _p)

import numpy as np
import ml_dtypes
from contextlib import ExitStack

import concourse.bass as bass
import concourse.mybir as mybir
import concourse.tile as tile
from concourse import bacc
from concourse.bass_utils import run_bass_kernel_spmd

BF16 = mybir.dt.bfloat16
F32 = mybir.dt.float32
NPBF16 = ml_dtypes.bfloat16

B, L, D, H = 2, 2048, 1024, 16
DH = D // H  # 64
NCORES = 8
HPC = H // NCORES  # heads per core = 2
DLOC = HPC * DH  # 128, local feature slice width
ESL = D // NCORES  # 128, output-feature slice per core


def build_attention_bass(b=B, seq=L, d=D, dh=DH):
    """Build the SPMD Bass program. Parameterized so a scaled-down config can
    be validated in CoreSim. Requires d % 128 == 0, seq % 128 == 0."""
    T = b * seq  # total tokens
    KD = d // 128  # contraction chunks over model dim
    dloc = 2 * dh  # per-core feature width (2 heads)
    esl = d // NCORES  # per-core output-feature slice
    scale = 1.0 / np.sqrt(dh)

    nc = bacc.Bacc(
        "TRN2",
        target_bir_lowering=False,
        debug=False,
        num_devices=NCORES,
    )

    # ---- DRAM I/O ----
    xT_d = nc.dram_tensor("xT", [d, T], BF16, kind="ExternalInput")
    wq_d = nc.dram_tensor("wq", [d, dloc], BF16, kind="ExternalInput")
    wk_d = nc.dram_tensor("wk", [d, dloc], BF16, kind="ExternalInput")
    wv_d = nc.dram_tensor("wv", [d, dloc], BF16, kind="ExternalInput")
    bq_d = nc.dram_tensor("bq", [dloc, 1], F32, kind="ExternalInput")
    bk_d = nc.dram_tensor("bk", [dloc, 1], F32, kind="ExternalInput")
    bv_d = nc.dram_tensor("bv", [1, dloc], BF16, kind="ExternalInput")
    wos_d = nc.dram_tensor("wos", [d, esl], BF16, kind="ExternalInput")
    bos_d = nc.dram_tensor("bos", [1, esl], BF16, kind="ExternalInput")
    out_d = nc.dram_tensor("out", [T, esl], F32, kind="ExternalOutput")

    with tile.TileContext(nc) as tc, ExitStack() as top:
        sb = top.enter_context(tc.tile_pool(name="sb", bufs=1))
        dram = top.enter_context(tc.tile_pool(name="dram", bufs=1, space="DRAM"))

        # ---- persistent SBUF tensors ----
        qt = sb.tile([dloc, T], BF16, tag="qt", name="qt")
        kt = sb.tile([dloc, T], BF16, tag="kt", name="kt")
        v_tiles = [
            sb.tile([128, dloc], BF16, tag=f"v{i}", name=f"v{i}")
            for i in range(T // 128)
        ]
        outT = sb.tile([dloc, T], BF16, tag="outT", name="outT")
        # weights as per-chunk tiles
        wq_t = [sb.tile([128, dloc], BF16, tag=f"wq{k}", name=f"wq{k}") for k in range(KD)]
        wk_t = [sb.tile([128, dloc], BF16, tag=f"wk{k}", name=f"wk{k}") for k in range(KD)]
        wv_t = [sb.tile([128, dloc], BF16, tag=f"wv{k}", name=f"wv{k}") for k in range(KD)]
        bq_s = sb.tile([dloc, 1], F32, tag="bq_s", name="bq_s")
        bk_s = sb.tile([dloc, 1], F32, tag="bk_s", name="bk_s")
        bv_s = sb.tile([1, dloc], BF16, tag="bv_s", name="bv_s")
        ones_s = sb.tile([1, 128], BF16, tag="ones_s", name="ones_s")
        bos_s = sb.tile([1, esl], BF16, tag="bos_s", name="bos_s")
        bo_bc = sb.tile([128, esl], F32, tag="bo_bc", name="bo_bc")

        # weight / bias loads
        for k in range(KD):
            nc.sync.dma_start(out=wq_t[k][:], in_=wq_d[k * 128:(k + 1) * 128, :])
            nc.sync.dma_start(out=wk_t[k][:], in_=wk_d[k * 128:(k + 1) * 128, :])
            nc.sync.dma_start(out=wv_t[k][:], in_=wv_d[k * 128:(k + 1) * 128, :])
        nc.sync.dma_start(out=bq_s[:], in_=bq_d[:])
        nc.sync.dma_start(out=bk_s[:], in_=bk_d[:])
        nc.sync.dma_start(out=bv_s[:], in_=bv_d[:])
        nc.sync.dma_start(out=bos_s[:], in_=bos_d[:])
        nc.vector.memset(ones_s[:], 1.0)

        with tc.tile_pool(name="xTpool", bufs=1) as xp:
            xt_t = [
                xp.tile([128, T], BF16, tag=f"xt{k}", name=f"xt{k}")
                for k in range(KD)
            ]
            for k in range(KD):
                nc.sync.dma_start(
                    out=xt_t[k][:], in_=xT_d[k * 128:(k + 1) * 128, :]
                )

            # ---- V = x @ Wv + bv   (token-partition layout) ----
            with tc.tile_pool(name="psv", bufs=3, space="PSUM") as psv:
                for ti in range(T // 128):
                    pv = psv.tile([128, dloc], F32, tag="pv", name="pv")
                    for k in range(KD):
                        nc.tensor.matmul(
                            pv[:],
                            lhsT=xt_t[k][:, ti * 128:(ti + 1) * 128],
                            rhs=wv_t[k][:],
                            start=(k == 0),
                            stop=False,
                        )
                    # + ones^T * bv  (broadcast bias along tokens)
                    nc.tensor.matmul(
                        pv[:],
                        lhsT=ones_s[:],
                        rhs=bv_s[:],
                        start=False,
                        stop=True,
                    )
                    nc.vector.tensor_scalar_add(v_tiles[ti][:], pv[:], 0.0)

                # bO broadcast tile (built once): ones^T @ bos
                pb = psv.tile([128, esl], F32, tag="pv", name="pb")
                nc.tensor.matmul(pb[:], lhsT=ones_s[:], rhs=bos_s[:],
                                 start=True, stop=True)
                nc.vector.tensor_scalar_add(bo_bc[:], pb[:], 0.0)

            # ---- Q^T, K^T projections + attention ----
            with tc.tile_pool(name="ps", bufs=2, space="PSUM") as ps, \
                 tc.tile_pool(name="pspv", bufs=1, space="PSUM") as pspv, \
                 tc.tile_pool(name="pp", bufs=4) as pp, \
                 tc.tile_pool(name="small", bufs=12) as sm:

                TCH = min(512, seq)  # token chunk for Q/K projection psum tiles
                for bb in range(b):
                    # Q^T/K^T for this batch's tokens
                    for t0 in range(bb * seq, (bb + 1) * seq, TCH):
                        for (wt, bt, dst) in ((wq_t, bq_s, qt), (wk_t, bk_s, kt)):
                            pq = ps.tile([128, 1024], F32, tag="s", name="pq")
                            for k in range(KD):
                                nc.tensor.matmul(
                                    pq[:dloc, :TCH],
                                    lhsT=wt[k][:],
                                    rhs=xt_t[k][:, t0:t0 + TCH],
                                    start=(k == 0),
                                    stop=(k == KD - 1),
                                )
                            # copy + per-partition bias + cast on DVE
                            nc.vector.tensor_scalar_add(
                                dst[:, t0:t0 + TCH], pq[:dloc, :TCH], bt[:]
                            )

                    # attention for batch bb, both local heads
                    nj = seq // 128
                    pvacc = pspv.tile([128, seq], F32, tag="pvacc", name="pvacc")
                    for jc in range(nj):
                        j0 = bb * seq + jc * 128
                        p_t = []
                        rcp_t = []
                        for h in range(2):
                            hs = h * dh
                            P = pp.tile([128, seq], BF16, tag="p", name="P")
                            lparts = []
                            for i0 in range(0, seq, 1024):
                                iw = min(1024, seq - i0)
                                S = ps.tile([128, 1024], F32, tag="s", name="S")
                                for s0 in range(0, iw, 512):
                                    sw = min(512, iw - s0)
                                    nc.tensor.matmul(
                                        S[:, s0:s0 + sw],
                                        lhsT=kt[hs:hs + dh, j0:j0 + 128],
                                        rhs=qt[hs:hs + dh,
                                               bb * seq + i0 + s0:
                                               bb * seq + i0 + s0 + sw],
                                        start=True,
                                        stop=True,
                                    )
                                lp = sm.tile([128, 1], F32, tag="lp", name="lp")
                                nc.scalar.activation(
                                    P[:, i0:i0 + iw],
                                    S[:, :iw],
                                    mybir.ActivationFunctionType.Exp,
                                    scale=float(scale),
                                    accum_out=lp[:],
                                )
                                lparts.append(lp)
                            if len(lparts) > 1:
                                lsum = sm.tile([128, 1], F32, tag="lp", name="lsum")
                                nc.vector.tensor_add(lsum[:], lparts[0][:], lparts[1][:])
                                for extra in lparts[2:]:
                                    nc.vector.tensor_add(lsum[:], lsum[:], extra[:])
                            else:
                                lsum = lparts[0]
                            rcp = sm.tile([128, 1], F32, tag="lp", name="rcp")
                            nc.vector.reciprocal(rcp[:], lsum[:])
                            p_t.append(P)
                            rcp_t.append(rcp)

                        # scale V rows by 1/l (per head half) then PV matmuls
                        vs = sm.tile([128, dloc], BF16, tag="vs", name="vs")
                        vt = v_tiles[j0 // 128]
                        for h in range(2):
                            hs = h * dh
                            nc.vector.tensor_scalar_mul(
                                vs[:, hs:hs + dh], vt[:, hs:hs + dh], rcp_t[h][:]
                            )
                        for h in range(2):
                            hs = h * dh
                            for s0 in range(0, seq, 512):
                                sw = min(512, seq - s0)
                                nc.tensor.matmul(
                                    pvacc[hs:hs + dh, s0:s0 + sw],
                                    lhsT=vs[:, hs:hs + dh],
                                    rhs=p_t[h][:, s0:s0 + sw],
                                    start=(jc == 0),
                                    stop=(jc == nj - 1),
                                    skip_group_check=True,
                                )
                    # write head outputs (d_local x seq) for this batch
                    nc.vector.tensor_scalar_add(
                        outT[:, bb * seq:(bb + 1) * seq], pvacc[:], 0.0
                    )

        # ---- AllGather head outputs across cores ----
        ag_in = dram.tile([dloc, T], BF16, tag="ag_in", name="ag_in")
        ag_out = dram.tile([NCORES * dloc, T], BF16, tag="ag_out",
                           name="ag_out", addr_space="Shared")
        nc.sync.dma_start(out=ag_in[:], in_=outT[:])
        nc.gpsimd.collective_compute(
            "AllGather",
            mybir.AluOpType.bypass,
            replica_groups=[list(range(NCORES))],
            ins=[ag_in[:]],
            outs=[ag_out[:]],
        )

        # ---- WO projection: y[:, e_slice] for all tokens ----
        with tc.tile_pool(name="agp", bufs=1) as agp, \
             tc.tile_pool(name="wop", bufs=1) as wop, \
             tc.tile_pool(name="psy", bufs=4, space="PSUM") as psy, \
             tc.tile_pool(name="ysb", bufs=4) as ysb:
            ag_t = [
                agp.tile([128, T], BF16, tag=f"ag{k}", name=f"ag{k}")
                for k in range(KD)
            ]
            wo_t = [
                wop.tile([128, esl], BF16, tag=f"wo{k}", name=f"wo{k}")
                for k in range(KD)
            ]
            for k in range(KD):
                nc.sync.dma_start(
                    out=ag_t[k][:], in_=ag_out[k * 128:(k + 1) * 128, :]
                )
                nc.sync.dma_start(
                    out=wo_t[k][:], in_=wos_d[k * 128:(k + 1) * 128, :]
                )
            for ti in range(T // 128):
                py = psy.tile([128, esl], F32, tag="py", name="py")
                for k in range(KD):
                    nc.tensor.matmul(
                        py[:],
                        lhsT=ag_t[k][:, ti * 128:(ti + 1) * 128],
                        rhs=wo_t[k][:],
                        start=(k == 0),
                        stop=(k == KD - 1),
                    )
                y = ysb.tile([128, esl], F32, tag="y", name="y")
                nc.vector.tensor_add(y[:], py[:], bo_bc[:, :esl])
                nc.sync.dma_start(
                    out=out_d[ti * 128:(ti + 1) * 128, :], in_=y[:]
                )

    nc.compile()
    return nc


def make_in_maps(x, Wq, bq, Wk, bk, Wv, bv, WO, bO, b=B, seq=L, d=D, dh=DH):
    """Host-side sharding: per-core input dicts (numpy)."""
    T = b * seq
    dloc = 2 * dh
    esl = d // NCORES
    xT = np.ascontiguousarray(
        np.asarray(x, np.float32).reshape(T, d).T
    ).astype(NPBF16)
    in_maps = []
    for c in range(NCORES):
        h0, h1 = 2 * c, 2 * c + 1
        cat = lambda W: np.ascontiguousarray(
            np.concatenate([np.asarray(W[h0], np.float32),
                            np.asarray(W[h1], np.float32)], axis=1)
        )
        catb = lambda bias: np.concatenate(
            [np.asarray(bias[h0], np.float32), np.asarray(bias[h1], np.float32)]
        )
        wos = np.ascontiguousarray(
            np.asarray(WO, np.float32).T[:, c * esl:(c + 1) * esl]
        )
        in_maps.append({
            "xT": xT,
            "wq": cat(Wq).astype(NPBF16),
            "wk": cat(Wk).astype(NPBF16),
            "wv": cat(Wv).astype(NPBF16),
            "bq": catb(bq).reshape(dloc, 1).astype(np.float32),
            "bk": catb(bk).reshape(dloc, 1).astype(np.float32),
            "bv": catb(bv).reshape(1, dloc).astype(NPBF16),
            "wos": wos.astype(NPBF16),
            "bos": np.asarray(bO, np.float32)[c * esl:(c + 1) * esl]
                     .reshape(1, esl).astype(NPBF16),
        })
    return in_maps


_NC_CACHE = {}


def kernel(x, Wq, bq, Wk, bk, Wv, bv, WO, bO):
    key = x.shape
    if key not in _NC_CACHE:
        _NC_CACHE[key] = build_attention_bass()
    nc = _NC_CACHE[key]
    in_maps = make_in_maps(x, Wq, bq, Wk, bk, Wv, bv, WO, bO)
    res = run_bass_kernel_spmd(nc, in_maps, core_ids=list(range(NCORES)))
    shards = [np.asarray(res.results[c]["out"], np.float32) for c in range(NCORES)]
    y = np.concatenate(shards, axis=1)  # [T, D]
    return y.reshape(B, L, D)


if __name__ == "__main__":
    import reference

    inputs = reference.setup_inputs()
    inputs = {k: np.asarray(v) for k, v in inputs.items()}
    expected = np.asarray(reference.reference(**inputs))
    actual = kernel(**inputs)
    err = np.linalg.norm(actual - expected) / np.linalg.norm(expected)
    print("Relative error:", err)
